# revision 1
# baseline (speedup 1.0000x reference)
"""Trainium2 Bass kernel for nn_AttentionCroiseeVariables.

Reference computation (N=4 vars, B=4, T=512, D=512, H=8, DK=DV=64):
  q,k,v = per-var projections of x; all-pairs (q_var, k_var) attention with
  per-key-var softmax; per-pair output projection; mean over key vars;
  residual + LayerNorm.

Sharding: 8 cores = (B=4) x (T split in 2 halves of 256 query tokens).
Core ci handles b = ci // 2, query-token half th = ci % 2.  Each core
computes its queries' attention over ALL key/value vars at full T=512
(K/V projections are recomputed by the 2 cores sharing a batch: +13.7us
PE, in exchange for zero cross-core communication).

On-chip layouts (bf16 compute, f32 accumulation):
  xt  [D=512, v*T=2048]      x transposed (host-prepped)
  qT  [hdk, 1024]            4 chunks [128, 1024], head-pair per chunk
  kT  [hdk, 2048]            4 chunks [128, 2048]
  V   [tok, hdv]             16 chunks [128, 512]
  scoresT[s, t] per (qv, c, head-pair): PSUM [128, 2048], row-tiled
    matmul pairs (two heads concurrent, K=dk=64 each)
  exp -> SBUF bf16 in one ACT call per block (the exp is the ACT-engine
    bottleneck: 64 x [128,2048] calls)
  denominators: ones[128,64] matmuls, col-tiled pairs -> PSUM [128,256]
    partition-REPLICATED sums; reciprocal + multiply normalize ctx
  AV: V-chunk as stationary [128,64], col-tiled head pairs -> ctxT pair
    [hdv-pair 128, t 256] = exactly the out-projection stationary layout
  out-proj: ctxT chunks x Wo chunks accumulate over (c, head-chunk) in
    PSUM [t 128, D 512]; then res = 0.25*out + x(+bo), LayerNorm with
    deferred sqrt batch (avoids exp<->sqrt ACT table thrash).
"""

import sys

import numpy as np

try:
    import concourse.bass as bass  # noqa: F401
except Exception:  # pragma: no cover
    sys.path.insert(0, "/opt/trn_rl_repo")

import ml_dtypes

import concourse.bass as bass
import concourse.tile as tile
from concourse import bacc, mybir
from concourse.bass_utils import run_bass_kernel_spmd

BF = mybir.dt.bfloat16
F32 = mybir.dt.float32
AF = mybir.ActivationFunctionType
OP = mybir.AluOpType

N, B, T, D = 4, 4, 512, 512
H, DK, DV = 8, 64, 64
TH = T // 2          # query tokens per core
NTOK = N * T         # kv tokens per core (all vars, one batch)
LN_EPS = 1e-5
SCALE = 1.0 / np.sqrt(DK)

_NC_CACHE = {}


def _dram_bcast_ap(handle, parts):
    """[parts, len] AP reading a 1-D DRAM tensor broadcast across partitions."""
    ap = handle[:]
    return bass.AP(tensor=ap.tensor, offset=ap.offset, ap=[[0, parts]] + list(ap.ap))


def build_nc():
    nc = bacc.Bacc(None, target_bir_lowering=False)

    xt_d = nc.dram_tensor("xt", [D, NTOK], BF, kind="ExternalInput")
    xq_d = nc.dram_tensor("xq", [D, N * TH], BF, kind="ExternalInput")
    xres_d = nc.dram_tensor("xres", [N * TH, D], F32, kind="ExternalInput")
    wq_d = nc.dram_tensor("wq", [D, H * DK], BF, kind="ExternalInput")
    wk_d = nc.dram_tensor("wk", [D, H * DK], BF, kind="ExternalInput")
    wv_d = nc.dram_tensor("wv", [D, H * DV], BF, kind="ExternalInput")
    wo_d = nc.dram_tensor("wo", [H * DV, D], BF, kind="ExternalInput")
    bq_d = nc.dram_tensor("bq", [H * DK], F32, kind="ExternalInput")
    bk_d = nc.dram_tensor("bk", [H * DK], F32, kind="ExternalInput")
    bv_d = nc.dram_tensor("bv", [H * DV], F32, kind="ExternalInput")
    bo_d = nc.dram_tensor("bo", [D], F32, kind="ExternalInput")
    gamma_d = nc.dram_tensor("gamma", [D], F32, kind="ExternalInput")
    beta_d = nc.dram_tensor("beta", [D], F32, kind="ExternalInput")
    out_d = nc.dram_tensor("out", [N * TH, D], F32, kind="ExternalOutput")

    with tile.TileContext(nc) as tc:
        with (
            tc.tile_pool(name="const", bufs=1) as constp,
            tc.tile_pool(name="xt", bufs=1) as xtp,
            tc.tile_pool(name="wts", bufs=1) as wtsp,
            tc.tile_pool(name="qkv", bufs=1) as qkvp,
            tc.tile_pool(name="attn", bufs=6) as attnp,
            tc.tile_pool(name="ctx", bufs=2) as ctxp,
            tc.tile_pool(name="outs", bufs=1) as outsp,
            tc.tile_pool(name="fin", bufs=3) as finp,
            tc.tile_pool(name="ps_s", bufs=2, space="PSUM") as ps_s,
            tc.tile_pool(name="ps_d", bufs=1, space="PSUM") as ps_d,
            tc.tile_pool(name="ps_av", bufs=1, space="PSUM") as ps_av,
            tc.tile_pool(name="ps_big", bufs=2, space="PSUM") as ps_big,
        ):
            # ---- constants
            ones_sb = constp.tile([128, 64], BF)
            nc.vector.memset(ones_sb, 1.0)
            eps_sb = constp.tile([128, 1], F32)
            nc.vector.memset(eps_sb, LN_EPS)
            bq_sb = constp.tile([128, 4], F32)
            nc.sync.dma_start(out=bq_sb, in_=bq_d[:].rearrange("(c p) -> p c", p=128))
            bk_sb = constp.tile([128, 4], F32)
            nc.sync.dma_start(out=bk_sb, in_=bk_d[:].rearrange("(c p) -> p c", p=128))
            bv_sb = constp.tile([128, H * DV], F32)
            nc.sync.dma_start(out=bv_sb, in_=_dram_bcast_ap(bv_d, 128))
            bo_sb = constp.tile([128, D], F32)
            nc.sync.dma_start(out=bo_sb, in_=_dram_bcast_ap(bo_d, 128))
            gamma_sb = constp.tile([128, D], F32)
            nc.sync.dma_start(out=gamma_sb, in_=_dram_bcast_ap(gamma_d, 128))
            beta_sb = constp.tile([128, D], F32)
            nc.sync.dma_start(out=beta_sb, in_=_dram_bcast_ap(beta_d, 128))

            # ---- bulk loads (spread across DMA issuers for parallelism)
            wq_sb, wk_sb, wv_sb, wo_sb = [], [], [], []
            for dj in range(4):
                for lst, dram, nm, eng in (
                    (wq_sb, wq_d, "wq", nc.scalar),
                    (wk_sb, wk_d, "wk", nc.scalar),
                    (wv_sb, wv_d, "wv", nc.gpsimd),
                    (wo_sb, wo_d, "wo", nc.gpsimd),
                ):
                    t_ = wtsp.tile([128, 512], BF, tag=f"{nm}{dj}")
                    eng.dma_start(out=t_, in_=dram[128 * dj : 128 * (dj + 1), :])
                    lst.append(t_)
            xt_sb, xq_sb = [], []
            for dj in range(4):
                t_ = xtp.tile([128, N * TH], BF, tag=f"xq{dj}")
                nc.scalar.dma_start(out=t_, in_=xq_d[128 * dj : 128 * (dj + 1), :])
                xq_sb.append(t_)
            for dj in range(4):
                t_ = xtp.tile([128, NTOK], BF, tag=f"xt{dj}")
                nc.sync.dma_start(out=t_, in_=xt_d[128 * dj : 128 * (dj + 1), :])
                xt_sb.append(t_)
            xres_sb = []
            for r in range(8):
                t_ = outsp.tile([128, D], F32, tag=f"xres{r}")
                nc.gpsimd.dma_start(out=t_, in_=xres_d[128 * r : 128 * (r + 1), :])
                xres_sb.append(t_)

            # ---- attention blocks + out-projection
            # qv-PAIRS: scores/exp per qv (PSUM-bank limited, N=256), but the
            # exp output for two qv's lands in one [128, 2048] attn tile per
            # head so dens/AV matmuls run at N=512 (half the instructions).
            # Emission is dependency-progressive: block (c, j) is emitted as
            # soon as projections for max(c, j) are out, so the ACT engine
            # starts exp work ~15us in instead of ~58us.
            res_tiles, mv_tiles = [], []

            def emit_block(qvp, c, j, ctx_tiles):
                attn_h = [
                    attnp.tile([128, 2, 4, 256], BF, tag="attn", name="a0"),
                    attnp.tile([128, 2, 4, 256], BF, tag="attn", name="a1"),
                ]
                for qh in range(2):
                    qv = 2 * qvp + qh
                    s_h = [
                        ps_s.tile([128, 1024], F32, tag="s", name="s0"),
                        ps_s.tile([128, 1024], F32, tag="s", name="s1"),
                    ]
                    for sc in range(4):
                        for h in range(2):
                            nc.tensor.matmul(
                                s_h[h][:, 256 * sc : 256 * (sc + 1)],
                                kt_sb[j][
                                    64 * h : 64 * (h + 1),
                                    512 * c + 128 * sc : 512 * c + 128 * (sc + 1),
                                ],
                                qt_sb[j][
                                    64 * h : 64 * (h + 1),
                                    256 * qv : 256 * (qv + 1),
                                ],
                                start=True,
                                stop=True,
                            )
                    for h in range(2):
                        nc.scalar.activation(
                            attn_h[h][:, qh], s_h[h], AF.Exp, scale=float(SCALE)
                        )
                d_ps = ps_d.tile([128, 512], F32, tag="d", name="d")
                for h in range(2):
                    for sc in range(4):
                        nc.tensor.matmul(
                            d_ps[64 * h : 64 * (h + 1), :],
                            ones_sb,
                            attn_h[h][:, :, sc, :],
                            start=(sc == 0),
                            stop=(sc == 3),
                        )
                av_ps = ps_av.tile([128, 512], F32, tag="av", name="av")
                for h in range(2):
                    for sc in range(4):
                        nc.tensor.matmul(
                            av_ps[64 * h : 64 * (h + 1), :],
                            v_sb[4 * c + sc][
                                :, 64 * (2 * j + h) : 64 * (2 * j + h + 1)
                            ],
                            attn_h[h][:, :, sc, :],
                            start=(sc == 0),
                            stop=(sc == 3),
                        )
                rb = attnp.tile([128, 512], F32, tag="rb", name="rb")
                nc.vector.reciprocal_approx_fast(rb, d_ps)
                ctx = ctxp.tile([128, 512], BF, tag=f"ctx{c}_{j}", name="ctx")
                nc.vector.tensor_tensor(ctx, av_ps, rb, OP.mult)
                ctx_tiles[(c, j)] = ctx

            def emit_outproj_ln(qvp, ctx_tiles):
                for qh in range(2):
                    qv = 2 * qvp + qh
                    for tch in range(2):
                        o_ps = ps_big.tile([128, 512], F32, tag="big", name="o")
                        nmm = 0
                        for c in range(N):
                            for j in range(4):
                                nmm += 1
                                nc.tensor.matmul(
                                    o_ps,
                                    ctx_tiles[(c, j)][
                                        :,
                                        256 * qh + 128 * tch : 256 * qh + 128 * (tch + 1),
                                    ],
                                    wo_sb[j],
                                    start=(nmm == 1),
                                    stop=(nmm == 16),
                                )
                        r = 2 * qv + tch
                        res = outsp.tile([128, D], F32, tag=f"res{r}", name="res")
                        nc.vector.scalar_tensor_tensor(
                            res, o_ps, 1.0 / N, xres_sb[r], OP.mult, OP.add
                        )
                        nc.vector.tensor_tensor(res, res, bo_sb, OP.add)
                        stats = finp.tile([128, 6], F32, tag="stats", name="st")
                        nc.vector.bn_stats(stats, res)
                        mv = outsp.tile([128, 2], F32, tag=f"mv{r}", name="mv")
                        nc.vector.bn_aggr(mv, stats)
                        rstd = finp.tile([128, 1], F32, tag="rstd", name="rst")
                        nc.scalar.activation(
                            rstd, mv[:, 1:2], AF.Sqrt, bias=eps_sb
                        )
                        rstd2 = finp.tile([128, 1], F32, tag="rstd2", name="rs2")
                        nc.vector.reciprocal(rstd2, rstd)
                        y = finp.tile([128, D], F32, tag="y", name="y")
                        nc.vector.tensor_scalar(
                            y, res, mv[:, 0:1], rstd2, OP.subtract, OP.mult
                        )
                        y2 = finp.tile([128, D], F32, tag="y2", name="y2")
                        nc.vector.tensor_tensor(y2, y, gamma_sb, OP.mult)
                        y3 = finp.tile([128, D], F32, tag="y3", name="y3")
                        nc.vector.tensor_tensor(y3, y2, beta_sb, OP.add)
                        eng = (nc.sync, nc.scalar, nc.gpsimd)[r % 3]
                        eng.dma_start(
                            out=out_d[128 * r : 128 * (r + 1), :], in_=y3
                        )

            ctx0, ctx1 = {}, {}
            progressive = [
                (c, j) for jj in range(4) for (c, j) in
                [(a, b) for a in range(4) for b in range(4) if max(a, b) == jj]
            ]

            # ---- projections
            # qT chunks: [hdk-pair 128, qv-major tokens 1024]
            qt_sb, kt_sb = [], []
            v_sb = [None] * 16
            for j in range(4):
                qt = qkvp.tile([128, N * TH], BF, tag=f"qt{j}")
                qt_sb.append(qt)
                for g in range(2):  # var pairs (2 vars x 256 tokens = 512)
                    q_ps = ps_big.tile([128, 512], F32, tag="big")
                    for dj in range(4):
                        nc.tensor.matmul(
                            q_ps,
                            wq_sb[dj][:, 128 * j : 128 * (j + 1)],
                            xq_sb[dj][:, 512 * g : 512 * (g + 1)],
                            start=(dj == 0),
                            stop=(dj == 3),
                        )
                    nc.vector.tensor_scalar_add(
                        qt[:, 512 * g : 512 * (g + 1)], q_ps, bq_sb[:, j : j + 1]
                    )
                kt = qkvp.tile([128, NTOK], BF, tag=f"kt{j}")
                kt_sb.append(kt)
                for g in range(4):
                    k_ps = ps_big.tile([128, 512], F32, tag="big")
                    for dj in range(4):
                        nc.tensor.matmul(
                            k_ps,
                            wk_sb[dj][:, 128 * j : 128 * (j + 1)],
                            xt_sb[dj][:, 512 * g : 512 * (g + 1)],
                            start=(dj == 0),
                            stop=(dj == 3),
                        )
                    nc.vector.tensor_scalar_add(
                        kt[:, 512 * g : 512 * (g + 1)], k_ps, bk_sb[:, j : j + 1]
                    )
                # V chunks for kv-var j: [tok-chunk 128, hdv 512] — emitted
                # here so the first attention blocks unblock early.
                for m in range(4 * j, 4 * j + 4):
                    v_ps = ps_big.tile([128, 512], F32, tag="big")
                    for dj in range(4):
                        nc.tensor.matmul(
                            v_ps,
                            xt_sb[dj][:, 128 * m : 128 * (m + 1)],
                            wv_sb[dj],
                            start=(dj == 0),
                            stop=(dj == 3),
                        )
                    vt = qkvp.tile([128, 512], BF, tag=f"v{m}")
                    nc.vector.tensor_tensor(vt, v_ps, bv_sb, OP.add)
                    v_sb[m] = vt
                for (c_, j_) in progressive:
                    if max(c_, j_) == j:
                        emit_block(0, c_, j_, ctx0)

            emit_outproj_ln(0, ctx0)
            for (c_, j_) in progressive:
                emit_block(1, c_, j_, ctx1)
            emit_outproj_ln(1, ctx1)

    nc.compile()
    return nc


def get_nc():
    if "nc" not in _NC_CACHE:
        _NC_CACHE["nc"] = build_nc()
    return _NC_CACHE["nc"]


def make_in_maps(x, Wq, bq, Wk, bk, Wv, bv, Wo, bo, gamma, beta):
    bf = ml_dtypes.bfloat16
    x = np.asarray(x, np.float32)
    wq16 = np.asarray(Wq, np.float32).astype(bf)
    wk16 = np.asarray(Wk, np.float32).astype(bf)
    wv16 = np.asarray(Wv, np.float32).astype(bf)
    wo16 = np.asarray(Wo, np.float32).astype(bf)
    vecs = {
        "bq": np.ascontiguousarray(np.asarray(bq, np.float32)),
        "bk": np.ascontiguousarray(np.asarray(bk, np.float32)),
        "bv": np.ascontiguousarray(np.asarray(bv, np.float32)),
        "bo": np.ascontiguousarray(np.asarray(bo, np.float32)),
        "gamma": np.ascontiguousarray(np.asarray(gamma, np.float32)),
        "beta": np.ascontiguousarray(np.asarray(beta, np.float32)),
    }
    in_maps = []
    for ci in range(8):
        b, th = ci // 2, ci % 2
        xb = x[:, b]  # [N, T, D]
        xt = np.ascontiguousarray(
            xb.transpose(2, 0, 1).reshape(D, NTOK)
        ).astype(bf)
        xq = np.ascontiguousarray(
            xb[:, th * TH : (th + 1) * TH, :].transpose(2, 0, 1).reshape(D, N * TH)
        ).astype(bf)
        xres = np.ascontiguousarray(
            xb[:, th * TH : (th + 1) * TH, :].reshape(N * TH, D)
        )
        m = {
            "xt": xt,
            "xq": xq,
            "xres": xres,
            "wq": np.ascontiguousarray(wq16),
            "wk": np.ascontiguousarray(wk16),
            "wv": np.ascontiguousarray(wv16),
            "wo": np.ascontiguousarray(wo16),
        }
        m.update(vecs)
        in_maps.append(m)
    return in_maps


def assemble(results):
    out = np.empty((N, B, T, D), np.float32)
    for ci in range(8):
        b, th = ci // 2, ci % 2
        o = np.asarray(results[ci]["out"], np.float32).reshape(N, TH, D)
        out[:, b, th * TH : (th + 1) * TH, :] = o
    return out


def kernel(**inputs) -> np.ndarray:
    nc = get_nc()
    in_maps = make_in_maps(**inputs)
    res = run_bass_kernel_spmd(nc, in_maps, core_ids=list(range(8)), trace=False)
    return assemble(res.results)



# revision 2
# speedup vs baseline: 1.1533x; 1.1533x over previous
"""Trainium2 Bass kernel for nn_AttentionCroiseeVariables.

Reference computation (N=4 vars, B=4, T=512, D=512, H=8, DK=DV=64):
  q,k,v = per-var projections of x; all-pairs (q_var, k_var) attention with
  per-key-var softmax; per-pair output projection; mean over key vars;
  residual + LayerNorm.

Sharding: 8 cores = (B=4) x (T split in 2 halves of 256 query tokens).
Core ci handles b = ci // 2, query-token half th = ci % 2.  Each core
computes its queries' attention over ALL key/value vars at full T=512.

Engine budget per core (v2):
  PE:  proj 34us + scores 27us (row-tiled head pairs) + av+den 55us
       (col-tiled pairs, sc-major interleave) + outproj 7us (ctx
       pre-summed over key vars on DVE)  ~= 130us
  ACT: 128 exp calls [128,1024] ~= 142us -> bottleneck; some (qh,head)
       exp units are offloaded to DVE via a bf16 Schraudolph bit-trick
       (tensor_scalar -> int16 bits reinterpreted as bf16).
  DVE: qkv PSUM->SBUF moves, softmax recip+normalize, ctx accumulate,
       LayerNorm stats; zero-bias / unit-gamma inputs compile to a
       specialized NEFF that skips the no-op adds/mults.

Emission is software-pipelined: a filler queue of proj/outproj matmul
groups is drained while the PE would otherwise stall waiting on exp.
"""

import sys
from collections import deque

import numpy as np

try:
    import concourse.bass as bass  # noqa: F401
except Exception:  # pragma: no cover
    sys.path.insert(0, "/opt/trn_rl_repo")

import ml_dtypes

import concourse.bass as bass
import concourse.tile as tile
from concourse import bacc, mybir
from concourse.bass_utils import run_bass_kernel_spmd

BF = mybir.dt.bfloat16
F32 = mybir.dt.float32
I16 = mybir.dt.int16
AF = mybir.ActivationFunctionType
OP = mybir.AluOpType

N, B, T, D = 4, 4, 512, 512
H, DK, DV = 8, 64, 64
TH = T // 2          # query tokens per core
NTOK = N * T         # kv tokens per core (all vars, one batch)
LN_EPS = 1e-5
SCALE = 1.0 / np.sqrt(DK)

# Schraudolph exp in bf16-bit space: exp(SCALE*x) ~= bitcast_bf16(
#   int16(round(x * SCALE*128*log2(e) + (127<<7) - 5.51)))
SCH_A = float(SCALE * 128.0 * np.log2(np.e))
SCH_B = float(127 * 128 - 5.51)
# exp units (qvp, c, j, qh, h) offloaded to DVE. Chosen in the late
# (qvp=1) phase where the PE has idle slack and ACT is the bottleneck.
N_SCHRAUD = 16

_NC_CACHE = {}


def _dram_bcast_ap(handle, parts):
    """[parts, len] AP reading a 1-D DRAM tensor broadcast across partitions."""
    ap = handle[:]
    return bass.AP(tensor=ap.tensor, offset=ap.offset, ap=[[0, parts]] + list(ap.ap))


def _schraud_units():
    units = set()
    cnt = 0
    for c in (3, 2, 1):
        for j in range(4):
            for (qh, h) in ((1, 1), (1, 0), (0, 1), (0, 0)):
                if cnt < N_SCHRAUD and (c >= 1):
                    units.add((1, c, j, qh, h))
                    cnt += 1
    return units


def build_nc(zb_q=True, zb_k=True, zb_v=True, zb_o=True, g1=True, zbeta=True):
    nc = bacc.Bacc(None, target_bir_lowering=False)

    xt_d = nc.dram_tensor("xt", [D, NTOK], BF, kind="ExternalInput")
    xq_d = nc.dram_tensor("xq", [D, N * TH], BF, kind="ExternalInput")
    xres_d = nc.dram_tensor("xres", [N * TH, D], BF, kind="ExternalInput")
    wq_d = nc.dram_tensor("wq", [D, H * DK], BF, kind="ExternalInput")
    wk_d = nc.dram_tensor("wk", [D, H * DK], BF, kind="ExternalInput")
    wv_d = nc.dram_tensor("wv", [D, H * DV], BF, kind="ExternalInput")
    wo_d = nc.dram_tensor("wo", [H * DV, D], BF, kind="ExternalInput")
    bq_d = nc.dram_tensor("bq", [H * DK], F32, kind="ExternalInput")
    bk_d = nc.dram_tensor("bk", [H * DK], F32, kind="ExternalInput")
    bv_d = nc.dram_tensor("bv", [H * DV], F32, kind="ExternalInput")
    bo_d = nc.dram_tensor("bo", [D], F32, kind="ExternalInput")
    gamma_d = nc.dram_tensor("gamma", [D], F32, kind="ExternalInput")
    beta_d = nc.dram_tensor("beta", [D], F32, kind="ExternalInput")
    out_d = nc.dram_tensor("out", [N * TH, D], F32, kind="ExternalOutput")

    schraud = _schraud_units()

    with tile.TileContext(nc) as tc:
        with (
            tc.tile_pool(name="const", bufs=1) as constp,
            tc.tile_pool(name="xt", bufs=1) as xtp,
            tc.tile_pool(name="wts", bufs=1) as wtsp,
            tc.tile_pool(name="qkv", bufs=1) as qkvp,
            tc.tile_pool(name="attn", bufs=6) as attnp,
            tc.tile_pool(name="ctx", bufs=1) as ctxp,
            tc.tile_pool(name="tmpc", bufs=3) as tmpcp,
            tc.tile_pool(name="outs", bufs=1) as outsp,
            tc.tile_pool(name="fin", bufs=3) as finp,
            tc.tile_pool(name="ps_s", bufs=2, space="PSUM") as ps_s,
            tc.tile_pool(name="ps_d", bufs=1, space="PSUM") as ps_d,
            tc.tile_pool(name="ps_av", bufs=1, space="PSUM") as ps_av,
            tc.tile_pool(name="ps_big", bufs=2, space="PSUM") as ps_big,
        ):
            # ---- constants
            ones_sb = constp.tile([128, 64], BF)
            nc.vector.memset(ones_sb, 1.0)
            eps_sb = constp.tile([128, 1], F32)
            nc.vector.memset(eps_sb, LN_EPS)
            dummy_sb = constp.tile([128, 1], F32)
            # warm the exp table set during the initial DMA wait
            nc.scalar.activation(dummy_sb, eps_sb, AF.Exp)

            if not (zb_q and zb_k):
                bq_sb = constp.tile([128, 4], F32)
                nc.sync.dma_start(out=bq_sb, in_=bq_d[:].rearrange("(c p) -> p c", p=128))
                bk_sb = constp.tile([128, 4], F32)
                nc.sync.dma_start(out=bk_sb, in_=bk_d[:].rearrange("(c p) -> p c", p=128))
            if not zb_v:
                bv_sb = constp.tile([128, H * DV], F32)
                nc.sync.dma_start(out=bv_sb, in_=_dram_bcast_ap(bv_d, 128))
            if not zb_o:
                bo_sb = constp.tile([128, D], F32)
                nc.sync.dma_start(out=bo_sb, in_=_dram_bcast_ap(bo_d, 128))
            if not g1:
                gamma_sb = constp.tile([128, D], F32)
                nc.sync.dma_start(out=gamma_sb, in_=_dram_bcast_ap(gamma_d, 128))
            if not zbeta:
                beta_sb = constp.tile([128, D], F32)
                nc.sync.dma_start(out=beta_sb, in_=_dram_bcast_ap(beta_d, 128))

            # ---- bulk loads, ordered so var-0 work can start ASAP
            wq_sb, wk_sb, wv_sb, wo_sb = [], [], [], []
            for dj in range(4):
                t_ = wtsp.tile([128, 512], BF, tag=f"wq{dj}", name="wq")
                nc.scalar.dma_start(out=t_, in_=wq_d[128 * dj : 128 * (dj + 1), :])
                wq_sb.append(t_)
                t_ = wtsp.tile([128, 512], BF, tag=f"wk{dj}", name="wk")
                nc.sync.dma_start(out=t_, in_=wk_d[128 * dj : 128 * (dj + 1), :])
                wk_sb.append(t_)
            # x transposed, one tile per (D-chunk, var): var-major order
            xtv = [[None] * 4 for _ in range(4)]  # [dj][v]
            for dj in range(4):
                t_ = xtp.tile([128, 512], BF, tag=f"xt{dj}_0", name="xt0")
                nc.gpsimd.dma_start(out=t_, in_=xt_d[128 * dj : 128 * (dj + 1), 0:512])
                xtv[dj][0] = t_
            for dj in range(4):
                t_ = wtsp.tile([128, 512], BF, tag=f"wv{dj}", name="wv")
                nc.scalar.dma_start(out=t_, in_=wv_d[128 * dj : 128 * (dj + 1), :])
                wv_sb.append(t_)
            xq_sb = []
            for dj in range(4):
                t_ = xtp.tile([128, N * TH], BF, tag=f"xq{dj}", name="xq")
                nc.sync.dma_start(out=t_, in_=xq_d[128 * dj : 128 * (dj + 1), :])
                xq_sb.append(t_)
            for v in range(1, 4):
                eng = (nc.gpsimd, nc.sync, nc.scalar)[v - 1]
                for dj in range(4):
                    t_ = xtp.tile([128, 512], BF, tag=f"xt{dj}_{v}", name="xtv")
                    eng.dma_start(
                        out=t_,
                        in_=xt_d[128 * dj : 128 * (dj + 1), 512 * v : 512 * (v + 1)],
                    )
                    xtv[dj][v] = t_
            for dj in range(4):
                t_ = wtsp.tile([128, 512], BF, tag=f"wo{dj}", name="wo")
                nc.gpsimd.dma_start(out=t_, in_=wo_d[128 * dj : 128 * (dj + 1), :])
                wo_sb.append(t_)
            xres_sb = []
            for r in range(8):
                t_ = outsp.tile([128, D], BF, tag=f"xres{r}", name="xres")
                nc.gpsimd.dma_start(out=t_, in_=xres_d[128 * r : 128 * (r + 1), :])
                xres_sb.append(t_)

            # ---- tiles that persist across phases
            qt_sb = [qkvp.tile([128, N * TH], BF, tag=f"qt{j}", name="qt") for j in range(4)]
            kt_sb = [qkvp.tile([128, NTOK], BF, tag=f"kt{j}", name="kt") for j in range(4)]
            v_sb = [qkvp.tile([128, 512], BF, tag=f"v{m}", name="vm") for m in range(16)]
            ctx_sum = {}

            filler = deque()

            def fill(n):
                for _ in range(min(n, len(filler))):
                    filler.popleft()()

            def drain_fill():
                while filler:
                    filler.popleft()()

            # ---- projection emitters
            def emit_qt(j, g):
                q_ps = ps_big.tile([128, 512], F32, tag="big", name="qps")
                for dj in range(4):
                    nc.tensor.matmul(
                        q_ps,
                        wq_sb[dj][:, 128 * j : 128 * (j + 1)],
                        xq_sb[dj][:, 512 * g : 512 * (g + 1)],
                        start=(dj == 0),
                        stop=(dj == 3),
                    )
                dst = qt_sb[j][:, 512 * g : 512 * (g + 1)]
                if zb_q:
                    nc.vector.tensor_copy(dst, q_ps)
                else:
                    nc.vector.tensor_scalar_add(dst, q_ps, bq_sb[:, j : j + 1])

            def emit_kt(j, g):
                k_ps = ps_big.tile([128, 512], F32, tag="big", name="kps")
                for dj in range(4):
                    nc.tensor.matmul(
                        k_ps,
                        wk_sb[dj][:, 128 * j : 128 * (j + 1)],
                        xtv[dj][g],
                        start=(dj == 0),
                        stop=(dj == 3),
                    )
                dst = kt_sb[j][:, 512 * g : 512 * (g + 1)]
                if zb_k:
                    nc.vector.tensor_copy(dst, k_ps)
                else:
                    nc.vector.tensor_scalar_add(dst, k_ps, bk_sb[:, j : j + 1])

            def emit_v(m):
                v_ps = ps_big.tile([128, 512], F32, tag="big", name="vps")
                for dj in range(4):
                    nc.tensor.matmul(
                        v_ps,
                        xtv[dj][m // 4][:, 128 * (m % 4) : 128 * (m % 4 + 1)],
                        wv_sb[dj],
                        start=(dj == 0),
                        stop=(dj == 3),
                    )
                if zb_v:
                    nc.vector.tensor_copy(v_sb[m], v_ps)
                else:
                    nc.vector.tensor_tensor(v_sb[m], v_ps, bv_sb, OP.add)

            # ---- attention block
            def emit_block(qvp, c, j):
                a_h = [
                    attnp.tile([128, 2, 4, 256], BF, tag="attn", name="a0"),
                    attnp.tile([128, 2, 4, 256], BF, tag="attn", name="a1"),
                ]
                for qh in range(2):
                    qv = 2 * qvp + qh
                    s_h = [
                        ps_s.tile([128, 1024], F32, tag="s", name="s0"),
                        ps_s.tile([128, 1024], F32, tag="s", name="s1"),
                    ]
                    for sc in range(4):
                        for h in range(2):
                            nc.tensor.matmul(
                                s_h[h][:, 256 * sc : 256 * (sc + 1)],
                                kt_sb[j][
                                    64 * h : 64 * (h + 1),
                                    512 * c + 128 * sc : 512 * c + 128 * (sc + 1),
                                ],
                                qt_sb[j][
                                    64 * h : 64 * (h + 1),
                                    256 * qv : 256 * (qv + 1),
                                ],
                                start=True,
                                stop=True,
                            )
                    for h in range(2):
                        if (qvp, c, j, qh, h) in schraud:
                            dst = a_h[h][:, qh].bitcast(I16).rearrange(
                                "p s t -> p (s t)"
                            )
                            nc.vector.tensor_scalar(
                                dst, s_h[h], SCH_A, SCH_B, OP.mult, OP.add
                            )
                        else:
                            nc.scalar.activation(
                                a_h[h][:, qh], s_h[h], AF.Exp, scale=float(SCALE)
                            )
                    fill(1)
                fill(1)
                d_ps = ps_d.tile([128, 512], F32, tag="d", name="d")
                for sc in range(4):
                    for h in range(2):
                        nc.tensor.matmul(
                            d_ps[64 * h : 64 * (h + 1), :],
                            ones_sb,
                            a_h[h][:, :, sc, :],
                            start=(sc == 0),
                            stop=(sc == 3),
                        )
                av_ps = ps_av.tile([128, 512], F32, tag="av", name="av")
                for sc in range(4):
                    for h in range(2):
                        nc.tensor.matmul(
                            av_ps[64 * h : 64 * (h + 1), :],
                            v_sb[4 * c + sc][
                                :, 64 * (2 * j + h) : 64 * (2 * j + h + 1)
                            ],
                            a_h[h][:, :, sc, :],
                            start=(sc == 0),
                            stop=(sc == 3),
                        )
                rb = attnp.tile([128, 512], F32, tag="rb", name="rb")
                nc.vector.reciprocal_approx_fast(rb, d_ps)
                key = (qvp, j)
                if key not in ctx_sum:
                    cs = ctxp.tile([128, 512], BF, tag=f"cs{qvp}_{j}", name="cs")
                    ctx_sum[key] = cs
                    nc.vector.tensor_tensor(cs, av_ps, rb, OP.mult)
                else:
                    tmp = tmpcp.tile([128, 512], BF, tag="tmp", name="tmp")
                    nc.vector.tensor_tensor(tmp, av_ps, rb, OP.mult)
                    nc.vector.tensor_tensor(ctx_sum[key], ctx_sum[key], tmp, OP.add)

            # ---- out-projection + LayerNorm stats (normalize deferred)
            res_tiles, mv_tiles = [None] * 8, [None] * 8

            def emit_outproj_group(qvp, qh, tch):
                qv = 2 * qvp + qh
                o_ps = ps_big.tile([128, 512], F32, tag="big", name="o")
                for j in range(4):
                    nc.tensor.matmul(
                        o_ps,
                        ctx_sum[(qvp, j)][
                            :, 256 * qh + 128 * tch : 256 * qh + 128 * (tch + 1)
                        ],
                        wo_sb[j],
                        start=(j == 0),
                        stop=(j == 3),
                    )
                r = 2 * qv + tch
                res = outsp.tile([128, D], F32, tag=f"res{r}", name="res")
                nc.vector.scalar_tensor_tensor(
                    res, o_ps, 1.0 / N, xres_sb[r], OP.mult, OP.add
                )
                if not zb_o:
                    nc.vector.tensor_tensor(res, res, bo_sb, OP.add)
                stats = finp.tile([128, 6], F32, tag="stats", name="st")
                nc.vector.bn_stats(stats, res)
                mv = outsp.tile([128, 2], F32, tag=f"mv{r}", name="mv")
                nc.vector.bn_aggr(mv, stats)
                res_tiles[r], mv_tiles[r] = res, mv

            # ================= emission schedule =================
            for j in range(4):
                emit_qt(j, 0)
            for j in range(4):
                emit_kt(j, 0)
            for m in range(4):
                emit_v(m)

            for v in range(4):
                if v < 3:
                    for j in range(4):
                        filler.append(lambda j=j, g=v + 1: emit_kt(j, g))
                    for m in range(4 * (v + 1), 4 * (v + 2)):
                        filler.append(lambda m=m: emit_v(m))
                if v == 0:
                    for j in range(4):
                        filler.append(lambda j=j: emit_qt(j, 1))
                for j in range(4):
                    emit_block(0, v, j)
                drain_fill()

            for qh in range(2):
                for tch in range(2):
                    filler.append(
                        lambda qh=qh, tch=tch: emit_outproj_group(0, qh, tch)
                    )
            for c in range(4):
                for j in range(4):
                    emit_block(1, c, j)
            drain_fill()
            for qh in range(2):
                for tch in range(2):
                    emit_outproj_group(1, qh, tch)

            # ---- deferred LayerNorm normalize (single sqrt table load)
            for r in range(8):
                rstd = finp.tile([128, 1], F32, tag="rstd", name="rst")
                nc.scalar.activation(rstd, mv_tiles[r][:, 1:2], AF.Sqrt, bias=eps_sb)
                rstd2 = finp.tile([128, 1], F32, tag="rstd2", name="rs2")
                nc.vector.reciprocal(rstd2, rstd)
                y = finp.tile([128, D], F32, tag="y", name="y")
                nc.vector.tensor_scalar(
                    y, res_tiles[r], mv_tiles[r][:, 0:1], rstd2, OP.subtract, OP.mult
                )
                if not g1:
                    nc.vector.tensor_tensor(y, y, gamma_sb, OP.mult)
                if not zbeta:
                    nc.vector.tensor_tensor(y, y, beta_sb, OP.add)
                eng = (nc.sync, nc.scalar, nc.gpsimd)[r % 3]
                eng.dma_start(out=out_d[128 * r : 128 * (r + 1), :], in_=y)

    nc.compile()
    return nc


def get_nc(spec=(True,) * 6):
    if spec not in _NC_CACHE:
        _NC_CACHE[spec] = build_nc(*spec)
    return _NC_CACHE[spec]


def input_spec(bq, bk, bv, bo, gamma, beta):
    return (
        not np.any(bq),
        not np.any(bk),
        not np.any(bv),
        not np.any(bo),
        bool(np.all(np.asarray(gamma, np.float32) == 1.0)),
        not np.any(beta),
    )


def make_in_maps(x, Wq, bq, Wk, bk, Wv, bv, Wo, bo, gamma, beta):
    bf = ml_dtypes.bfloat16
    x = np.asarray(x, np.float32)
    wq16 = np.ascontiguousarray(np.asarray(Wq, np.float32).astype(bf))
    wk16 = np.ascontiguousarray(np.asarray(Wk, np.float32).astype(bf))
    wv16 = np.ascontiguousarray(np.asarray(Wv, np.float32).astype(bf))
    wo16 = np.ascontiguousarray(np.asarray(Wo, np.float32).astype(bf))
    vecs = {
        "bq": np.ascontiguousarray(np.asarray(bq, np.float32)),
        "bk": np.ascontiguousarray(np.asarray(bk, np.float32)),
        "bv": np.ascontiguousarray(np.asarray(bv, np.float32)),
        "bo": np.ascontiguousarray(np.asarray(bo, np.float32)),
        "gamma": np.ascontiguousarray(np.asarray(gamma, np.float32)),
        "beta": np.ascontiguousarray(np.asarray(beta, np.float32)),
    }
    in_maps = []
    for ci in range(8):
        b, th = ci // 2, ci % 2
        xb = x[:, b]  # [N, T, D]
        xt = np.ascontiguousarray(
            xb.transpose(2, 0, 1).reshape(D, NTOK)
        ).astype(bf)
        xq = np.ascontiguousarray(
            xb[:, th * TH : (th + 1) * TH, :].transpose(2, 0, 1).reshape(D, N * TH)
        ).astype(bf)
        xres = np.ascontiguousarray(
            xb[:, th * TH : (th + 1) * TH, :].reshape(N * TH, D)
        ).astype(bf)
        m = {
            "xt": xt,
            "xq": xq,
            "xres": xres,
            "wq": wq16,
            "wk": wk16,
            "wv": wv16,
            "wo": wo16,
        }
        m.update(vecs)
        in_maps.append(m)
    return in_maps


def assemble(results):
    out = np.empty((N, B, T, D), np.float32)
    for ci in range(8):
        b, th = ci // 2, ci % 2
        o = np.asarray(results[ci]["out"], np.float32).reshape(N, TH, D)
        out[:, b, th * TH : (th + 1) * TH, :] = o
    return out


def kernel(**inputs) -> np.ndarray:
    spec = input_spec(
        inputs["bq"], inputs["bk"], inputs["bv"],
        inputs["bo"], inputs["gamma"], inputs["beta"],
    )
    nc = get_nc(spec)
    in_maps = make_in_maps(**inputs)
    res = run_bass_kernel_spmd(nc, in_maps, core_ids=list(range(8)), trace=False)
    return assemble(res.results)
